# revision 76
# baseline (speedup 1.0000x reference)
"""Self-contained Trainium2 kernel for the fused attention layer.

Reference semantics (B=4, N=2048, D=512, H=8, E=64):
    ln = LayerNorm(x) ; q/k/v/gate head projections ; RoPE (quirk: position
    index = HEAD index, so RoPE is a constant per-head orthogonal rotation
    that we fold into q_proj/k_proj on the host) ; masked softmax attention ;
    sigmoid gating ; output projection ; residual ; LayerNorm.

Sharding: 8 cores, core c -> (batch b = c//2, query-row half j = c%2).
Each core computes full K/V for its batch (duplicated across the 2 cores of
a batch -- cheaper than any collective) and attention + output projection +
final LN for its 1024 query rows.  Host rolls the rows of x so every core's
query rows are rows [0:1024) of its own input -> all 8 cores run an
identical SPMD graph with no per-core constants.

Masking scheme (no -1e9 bias anywhere): the layernormed activations of PAD
tokens are zeroed on device, so pad K columns and pad V rows are exactly 0,
pad scores are 0, and exp(-3)>0 is a constant that cancels between the
numerator and the mask-column denominator.  Pad query rows are zeroed by
folding the row mask into the sigmoid gate (adding 1e30 to 1+exp(-x)).

Perf notes (v3, ~248us vs the bf16 v2's ~256us):
  - PV matmuls run in fp8 DoubleRow mode: V in fp8e4 (stationary, two key
    chunks as the two DR k-tiles) x exp-scores in fp8e5 (moving).  One DR
    matmul covers a chunk PAIR (256-deep contraction) in ~380ns vs 2x274ns
    for the bf16 pair.  NOTE (measured): DR cost on HW ~= moving-column
    count x 1 cycle, i.e. the fp8 gain comes ONLY from doubling the
    contraction depth per column pass -- fp8 for the S matmuls (contraction
    E=64) would gain nothing, so S stays bf16.
  - exp() writes fp8e5 directly with a folded -6 bias (exp(s/8-6)); e5m2's
    57344 max cannot overflow (observed global max s/8 is ~15.3 -- e4m3's
    240/448 DOES overflow -> NaN).  The bias and quantization are common to
    numerator and denominator so softmax normalization cancels them.
  - V carries a ones*colmask column at col E, so PV psum row E is the
    softmax denominator; reciprocal_approx_fast (exact reciprocal on DVE is
    5x slower!) + gpsimd.partition_broadcast replicate it to partitions
    0:64 (no PE broadcast matmuls).
  - The row mask folds into the gate: gt2 = 1/((1+exp(-pg)) + (1-rm)*1e30)
    via one fused scalar_tensor_tensor (recip_approx_fast is documented
    safe below ~1e38).
  - Final-LN rstd uses a DVE fast-inverse-sqrt (bit trick + 2 Newton
    steps), so no mid-stream ACT sqrt-table reloads (each reload is 1.28us
    AND HAM-throttles the PE clock for tens of us) and no tail fence; the
    ns=0 half of the tail overlaps the ns=1 attention stream.
  - ACT table loads: exactly 2 (sqrt at ramp for phase-A rstd, exp right
    after, preloaded before the first score exp).
  - PSUM rings: scores 2x(2 banks), PV-accum/out-proj/phase-A-transposes
    2x1, projections/gates 2x1.  Keeping the phase-A transposes in psO is
    load-bearing: phase A needs 4 psum slots to pipeline transposes against
    projections (tried psM-only and half/half splits: both slower), even
    though it delays the first PV group until phase A drains.
  - Things measured SLOWER and reverted: DMA-XBAR transposes for lnT (1.2us
    each, serialized on the issuing queue's timeline, and they wedged the
    device once); per-quarter/per-half phase A (starves the S stream at
    quarter boundaries and drops PE to a lower p-state, +14us PE busy);
    pair-batched S/PV emission (lock-steps the 2-slot psS ring, +44us);
    deadline-ordered projection blocks (PE idles instead, pstate drop).
"""

import numpy as np

B, N, D, H, E = 4, 2048, 512, 8, 64
NR = N // 2            # query rows per core
P = 128                # partitions
DCH = D // P           # 4 d-chunks
MCH = N // P           # 16 m-chunks
MPAIR = MCH // 2       # 8 m-chunk pairs
MSEG = N // 512        # 4 key segments
NSEG = NR // 512       # 2 query segments
NTIL = NR // P         # 8 query row tiles
HP = H // 2            # head pairs
HE = H * E
VW = 96                # V tile width: cols 0:64 = v, col 64 = col mask,
#                        cols 65:96 zero padding (96 = aligned LDWEIGHTS width)
EPS = 1e-6
PAD = -2.0
SCALE = 1.0 / np.sqrt(E).astype(np.float32)
EXPB = -6.0            # constant bias inside exp(); cancels in normalization.
#                        e5m2 scores overflow at score*scale > 10.96 - EXPB;
#                        the observed global max score*scale is ~15.3.
RSQRT_MAGIC = 0x5F3759DF

_CACHE = {}


def _build_nc(trivial_affines=True, act_apply=True, bf16_tp=True,
              use_rmh=True, use_fast_rsqrt=True, debug=False,
              debug_parts=("S", "C", "G", "E")):
    import concourse.bass as bass
    import concourse.bacc as bacc
    import concourse.mybir as mybir
    from concourse.tile import TileContext
    from concourse.masks import make_identity
    from contextlib import ExitStack

    f32 = mybir.dt.float32
    i32 = mybir.dt.int32
    CDT = mybir.dt.bfloat16
    FP8V = mybir.dt.float8e4
    FP8P = mybir.dt.float8e5
    AF = mybir.ActivationFunctionType
    ALU = mybir.AluOpType
    DR = mybir.MatmulPerfMode.DoubleRow

    nc = bacc.Bacc()

    x_ext = nc.declare_dram_parameter("x", [N, D], f32, isOutput=False)
    wproj_ext = nc.declare_dram_parameter("wproj", [P, 4 * DCH * HE], CDT, isOutput=False)
    ow_ext = nc.declare_dram_parameter("ow", [P, DCH * D], CDT, isOutput=False)
    vecs_ext = nc.declare_dram_parameter("vecs", [5, D], f32, isOutput=False)
    cm_ext = nc.declare_dram_parameter("cm", [P, MCH], f32, isOutput=False)
    cmb_ext = nc.declare_dram_parameter("cmb", [P, MCH], CDT, isOutput=False)
    rmh_ext = nc.declare_dram_parameter("rmh", [NR], CDT, isOutput=False)
    rmb_ext = nc.declare_dram_parameter("rmb", [NR], CDT, isOutput=False)
    out_ext = nc.declare_dram_parameter("out", [NR, D], f32, isOutput=True)
    if debug:
        dbg = {
            "dbg_ss": nc.declare_dram_parameter("dbg_ss", [P, 1024], f32, isOutput=True),
            "dbg_pts": nc.declare_dram_parameter("dbg_pts", [P, 1024], FP8P, isOutput=True),
            "dbg_po": nc.declare_dram_parameter("dbg_po", [P, 512], f32, isOutput=True),
            "dbg_rrb": nc.declare_dram_parameter("dbg_rrb", [64, 512], f32, isOutput=True),
            "dbg_gt2": nc.declare_dram_parameter("dbg_gt2", [P, 512], f32, isOutput=True),
            "dbg_vp": nc.declare_dram_parameter("dbg_vp", [P, 2 * H * VW], FP8V, isOutput=True),
            "dbg_kt": nc.declare_dram_parameter("dbg_kt", [P, 512], CDT, isOutput=True),
            "dbg_ot2": nc.declare_dram_parameter("dbg_ot2", [P, 512], CDT, isOutput=True),
            "dbg_yt": nc.declare_dram_parameter("dbg_yt", [P, 512], f32, isOutput=True),
        }

    def bcast(ap2d, p=P):
        # replicate a (1, L) DRAM AP across p partitions via step-0 AP
        return bass.AP(tensor=ap2d.tensor, offset=ap2d.offset,
                       ap=[[0, p]] + list(ap2d.ap[1:]))

    def woff(proj, dc, h=0):
        return ((proj * DCH + dc) * H + h) * E

    with TileContext(nc) as tc, ExitStack() as ctx:
        const = ctx.enter_context(tc.tile_pool(name="const", bufs=1))
        stat = ctx.enter_context(tc.tile_pool(name="stat", bufs=8))
        ppt = ctx.enter_context(tc.tile_pool(name="ppt", bufs=10))
        otp = ctx.enter_context(tc.tile_pool(name="otp", bufs=2))
        xrp = ctx.enter_context(tc.tile_pool(name="xrp", bufs=4))
        psS = ctx.enter_context(tc.tile_pool(name="psS", bufs=2, space="PSUM"))
        psO = ctx.enter_context(tc.tile_pool(name="psO", bufs=2, space="PSUM"))
        psM = ctx.enter_context(tc.tile_pool(name="psM", bufs=2, space="PSUM"))

        # ---- persistent intermediates ----
        lnT = const.tile([P, DCH, N], CDT)        # ln(x)^T: [d%P, d//P, n]
        KT2 = const.tile([P, HP, N], CDT)         # [e + 64*(h%2), h//2, m]
        QT2 = const.tile([P, HP, NR], CDT)        # packed like KT2
        Vp = const.tile([P, MPAIR, 2, H, VW], FP8V)  # [m%P, pair, kt, h, e|mask]
        OT2 = const.tile([P, DCH, NR], CDT)       # [(h*64+e)%P, (h*64+e)//P, n]
        xq = const.tile([P, NTIL, D], f32)        # x rows 0:NR (residual+phaseA)
        yt_all = const.tile([P, NTIL, D], f32)    # pre-final-LN activations
        mv_all = const.tile([P, NTIL, 2], f32)    # final-LN mean/var per tile

        # ---- constants ----
        TDT = CDT if bf16_tp else f32
        ident = const.tile([P, P], TDT)
        make_identity(nc, ident)
        cm = const.tile([P, MCH], f32)
        nc.sync.dma_start(out=cm, in_=cm_ext[:, :])
        epsT = const.tile([P, 1], f32)
        nc.vector.memset(epsT, EPS)
        expbT = const.tile([P, 1], f32)
        nc.vector.memset(expbT, EXPB)
        # zero the V padding columns while the DVE is still idle (pre-x)
        nc.vector.memset(Vp[:, :, :, :, E + 1:VW], 0.0)
        # DMA queue order is ramp-critical: x tiles 0-3 first (stats chain
        # starts ~1us in), then the K/Q half of wproj (needed once transposes
        # of tiles 0-3 land), then more x, the V/gate half of wproj, and
        # finally ow (needed only much later at the out-projection).
        xrt = {}
        def xsrc(t0, t1):
            return x_ext[t0 * P:t1 * P, :].rearrange("(t p) d -> p t d", p=P)

        nc.sync.dma_start(out=xq[:, 0:4, :], in_=xsrc(0, 4))
        cmb = const.tile([P, MCH], CDT)
        nc.sync.dma_start(out=cmb, in_=cmb_ext[:, :])
        for t in range(4, NTIL):
            nc.sync.dma_start(out=xq[:, t, :], in_=x_ext[t * P:(t + 1) * P, :])
        wproj = const.tile([P, 4 * DCH * HE], CDT)
        nc.sync.dma_start(out=wproj[:, 0:2 * DCH * HE],
                          in_=wproj_ext[:, 0:2 * DCH * HE])
        rmh = const.tile([P, NR], CDT)            # (1-rowmask)*1e30, replicated
        nc.sync.dma_start(out=rmh, in_=bcast(rmh_ext[None, :]))
        rmb = const.tile([P, NR], CDT)            # rowmask 0/1, replicated
        nc.sync.dma_start(out=rmb, in_=bcast(rmb_ext[None, :]))
        for t in range(NTIL, MCH):
            xrt[t] = xrp.tile([P, D], f32, tag="xr", name="xr", bufs=8)
            nc.sync.dma_start(out=xrt[t], in_=x_ext[t * P:(t + 1) * P, :])
        nc.sync.dma_start(out=wproj[:, 2 * DCH * HE:],
                          in_=wproj_ext[:, 2 * DCH * HE:])
        ow = const.tile([P, DCH * D], CDT)
        nc.sync.dma_start(out=ow, in_=ow_ext[:, :])
        if not trivial_affines:
            gin = const.tile([P, D], f32)
            bin_ = const.tile([P, D], f32)
            gout = const.tile([P, D], f32)
            bout = const.tile([P, D], f32)
            obias = const.tile([P, D], f32)
            for i, t in enumerate([gin, bin_, gout, bout, obias]):
                nc.sync.dma_start(out=t, in_=bcast(vecs_ext[i:i + 1, :]))

        # ---- projection blocks (emitted when lnT inputs are ready) ----
        def b_K(hp, ms):
            pk = psM.tile([P, 512], f32, tag="m", name="pk")
            for dc in range(DCH):
                nc.tensor.matmul(pk,
                                 wproj[:, woff(1, dc, 2 * hp):woff(1, dc, 2 * hp) + 2 * E],
                                 lnT[:, dc, ms * 512:(ms + 1) * 512],
                                 start=(dc == 0), stop=(dc == DCH - 1))
            if hp == 0:
                # jump the ramp's DVE copy backlog: these copies gate the
                # first score matmuls
                with tc.high_priority():
                    nc.vector.tensor_copy(
                        out=KT2[:, hp, ms * 512:(ms + 1) * 512], in_=pk)
            else:
                nc.vector.tensor_copy(out=KT2[:, hp, ms * 512:(ms + 1) * 512],
                                      in_=pk)

        def b_Q(hp, ns):
            nsl = slice(ns * 512, (ns + 1) * 512)
            pq = psM.tile([P, 512], f32, tag="m", name="pq")
            for dc in range(DCH):
                nc.tensor.matmul(pq,
                                 wproj[:, woff(0, dc, 2 * hp):woff(0, dc, 2 * hp) + 2 * E],
                                 lnT[:, dc, nsl],
                                 start=(dc == 0), stop=(dc == DCH - 1))
            if hp == 0:
                with tc.high_priority():
                    nc.vector.tensor_copy(out=QT2[:, hp, nsl], in_=pq)
            else:
                nc.vector.tensor_copy(out=QT2[:, hp, nsl], in_=pq)

        def b_V(mc):
            pv = psM.tile([P, HE], f32, tag="m", name="pv")
            for dc in range(DCH):
                nc.tensor.matmul(pv, lnT[:, dc, mc * P:(mc + 1) * P],
                                 wproj[:, woff(2, dc):woff(2, dc) + HE],
                                 start=(dc == 0), stop=(dc == DCH - 1))
            nc.vector.tensor_copy(
                out=Vp[:, mc // 2, mc % 2, :, 0:E],
                in_=pv[:].rearrange("p (h e) -> p h e", e=E))

        # block queue ordered by CONSUMPTION deadline (the attention-stream
        # pair index that first needs each block), not by input readiness:
        # emitting a not-yet-needed hp1-3 block early parks ~1us of PE work
        # in front of the blocks the next S pairs are waiting on.
        # deadline: K(hp,ms) first used at stream pair hp*16 + 2*ms (ns0);
        # Q(hp,ns) at ns*64 + hp*16; V(mc) at PV pair mc//2 (+L lag).
        L = 6              # pairs the S/exp stream leads PV consumption by
        blocks = []
        for hp in range(HP):
            for ms in range(MSEG):
                blocks.append((b_K, (hp, ms), 4 * ms + 3, hp * 16 + 2 * ms))
            for ns in range(NSEG):
                blocks.append((b_Q, (hp, ns), 4 * ns + 3, ns * 64 + hp * 16))
            for mc in range(4 * hp, 4 * hp + 4):
                blocks.append((b_V, (mc,), mc, mc // 2 + L))
        def bkey(b):
            fn, args, need, _ = b
            if fn.__name__ == "b_V":
                grp = 1
            elif args[0] == 0:
                grp = 0
            else:
                grp = 1 + args[0]
            return (need, grp)
        blocks.sort(key=bkey)
        emitted = set()

        def pop_blocks(n, tdone):
            k = 0
            while k < n and blocks:
                fn, args, need, _ = blocks[0]
                if need > tdone:
                    break
                blocks.pop(0)
                fn(*args)
                emitted.add((fn.__name__, args))
                k += 1

        def need_block(fn, args):
            if (fn.__name__, args) in emitted:
                return
            for i, (f2, a2, _, _) in enumerate(blocks):
                if f2 is fn and a2 == args:
                    blocks.pop(i)
                    break
            fn(*args)
            emitted.add((fn.__name__, args))

        # ---- phase A: layernorm (pad rows zeroed) + transpose ----
        xts = []
        mvA = const.tile([P, MCH, 2], f32)
        rstdv = const.tile([P, MCH], f32)
        nmbA = const.tile([P, MCH], f32)
        HB = MCH // 2

        def pass2_tile(t):
            from contextlib import nullcontext
            xt = xts[t]
            hot = tc.high_priority() if t >= HB else nullcontext()
            lnf = otp.tile([P, D], TDT, tag="lnf", bufs=6)
            if act_apply and t < HB:
                nc.scalar.activation(out=lnf, in_=xt, func=AF.Identity,
                                     bias=nmbA[:, t:t + 1], scale=rstdv[:, t:t + 1])
            else:
                with hot:
                    nc.vector.tensor_scalar(out=lnf, in0=xt,
                                            scalar1=mvA[:, t, 0:1],
                                            scalar2=rstdv[:, t:t + 1],
                                            op0=ALU.subtract, op1=ALU.mult)
            if not trivial_affines:
                nc.vector.tensor_mul(lnf, lnf, gin)
                nc.vector.tensor_add(lnf, lnf, bin_)
                nc.vector.tensor_scalar_mul(lnf, lnf, cm[:, t:t + 1])
            for dc in range(DCH):
                pt = psO.tile([P, P], TDT, tag="o", name="pt")
                nc.tensor.transpose(pt, lnf[:, dc * P:(dc + 1) * P], ident)
                with (tc.high_priority() if t >= HB else nullcontext()):
                    nc.vector.tensor_copy(out=lnT[:, dc, t * P:(t + 1) * P],
                                          in_=pt)
            pop_blocks(2, t)

        # ---- DVE fast inverse sqrt (no ACT table): bit trick + 2 Newton ----
        magicT = const.tile([P, 1], i32)
        nc.vector.memset(magicT, RSQRT_MAGIC)

        def fast_rsqrt(dst, var_ap, n):
            # dst [P, n] f32 <- 1/sqrt(var_ap + EPS)
            v = stat.tile([P, n], f32, tag="ve", bufs=2)
            nc.vector.tensor_scalar_add(v, var_ap, EPS)
            ib = stat.tile([P, n], i32, tag="ib", bufs=2)
            nc.vector.tensor_scalar(out=ib, in0=v[:].bitcast(i32), scalar1=1,
                                    scalar2=None, op0=ALU.logical_shift_right)
            nc.vector.scalar_tensor_tensor(
                out=ib, in0=ib, scalar=-1, in1=magicT[:].broadcast_to((P, n)),
                op0=ALU.mult, op1=ALU.add)
            r = ib[:].bitcast(f32)
            a = stat.tile([P, n], f32, tag="nw", bufs=2)
            for _ in range(2):
                nc.vector.tensor_mul(a, r, r)
                nc.vector.tensor_mul(a, a, v)
                nc.vector.tensor_scalar(out=a, in0=a, scalar1=-0.5, scalar2=1.5,
                                        op0=ALU.mult, op1=ALU.add)
                nc.vector.tensor_mul(dst, r, a)
                r = dst[:]

        # all 16 stats first (DMA-paced; nothing else competes on DVE)
        for t in range(MCH):
            xt = xq[:, t, :] if t < NTIL else xrt[t]
            xts.append(xt)
            st = stat.tile([P, 6], f32, tag="st")
            nc.vector.bn_stats(out=st, in_=xt)
            nc.vector.bn_aggr(out=mvA[:, t, :], in_=st)
        for half in range(2):
            if half == 1:
                # V mask column: Vp[:, pair, kt, h, E] <- cmb[:, 2*pair+kt]
                # (one strided copy; cmb has landed by now)
                c2 = cmb[:, :]
                srcb = bass.AP(tensor=c2.tensor, offset=c2.offset,
                               ap=[list(c2.ap[0]),
                                   [c2.ap[1][0] * 2, MPAIR],
                                   [c2.ap[1][0], 2], [0, H]])
                nc.vector.tensor_copy(out=Vp[:, :, :, :, E], in_=srcb)
            hsl = slice(half * HB, (half + 1) * HB)
            nc.scalar.activation(out=rstdv[:, hsl], in_=mvA[:, hsl, 1],
                                 func=AF.Sqrt, bias=epsT, scale=1.0)
            nc.vector.reciprocal(out=rstdv[:, hsl], in_=rstdv[:, hsl])
            nc.vector.tensor_mul(rstdv[:, hsl], rstdv[:, hsl], cm[:, hsl])
            nc.vector.tensor_mul(nmbA[:, hsl], mvA[:, hsl, 0], rstdv[:, hsl])
            nc.vector.tensor_scalar_mul(nmbA[:, hsl], nmbA[:, hsl], -1.0)
            if half == 1:
                # preload the exp table set right after the last sqrt; the
                # rstdv read pins it here (dep-free ops get hoisted earlier)
                scr = stat.tile([P, 1], f32, tag="scr")
                nc.scalar.activation(out=scr, in_=rstdv[:, MCH - 1:MCH],
                                     func=AF.Exp)
            for t in range(half * HB, (half + 1) * HB):
                pass2_tile(t)

        # ---- phase C: attention (software-pipelined) ----
        # One flat stream over all (iter, pair) score blocks.  S-matmul+exp of
        # pair g is emitted L pairs ahead of its PV consumption.
        NDELAY = 5         # pairs between PV-group close and its norm ops
        iters = [(ns, hp, hr)
                 for ns in range(NSEG)
                 for hp in range(HP)
                 for hr in (0, 64)]
        NIT = len(iters)
        NG = NIT * MPAIR
        pts = {}           # live exp outputs: global pair index -> tile
        po_cur = [None]    # open PV psum group
        gate_cur = {}      # (ns, hp) -> (gt2, gts)
        pending_norm = []  # (close_pair, it, po, rrb)

        def emit_S(g):
            it, p = divmod(g, MPAIR)
            ns, hp, hr = iters[it]
            need_block(b_K, (hp, p // 2))
            need_block(b_Q, (hp, ns))
            nsl = slice(ns * 512, (ns + 1) * 512)
            ss = psS.tile([P, 1024], f32, tag="s", name="ss")
            for j in (0, 1):
                mc = 2 * p + j
                nc.tensor.matmul(ss[:, j * 512:(j + 1) * 512],
                                 KT2[hr:hr + 64, hp, mc * P:(mc + 1) * P],
                                 QT2[hr:hr + 64, hp, nsl],
                                 start=True, stop=True)
            ptc = ppt.tile([P, 1024], FP8P, tag="pt")
            nc.scalar.activation(out=ptc, in_=ss, func=AF.Exp,
                                 scale=float(SCALE), bias=expbT)
            pts[g] = ptc
            if debug and "S" in debug_parts and g == 0:
                sd = const.tile([P, 1024], f32)
                nc.vector.tensor_copy(out=sd, in_=ss)
                nc.sync.dma_start(out=dbg["dbg_ss"][:, :], in_=sd)
                nc.sync.dma_start(out=dbg["dbg_pts"][:, :], in_=ptc)

        def emit_gate(ns, hp):
            # sigmoid(x)*rowmask = 1/((1+exp(-x)) + (1-rm)*1e30)
            nsl = slice(ns * 512, (ns + 1) * 512)
            pg = psM.tile([P, 512], f32, tag="m", name="pg")
            for dc in range(DCH):
                nc.tensor.matmul(pg,
                                 wproj[:, woff(3, dc, 2 * hp):woff(3, dc, 2 * hp) + 2 * E],
                                 lnT[:, dc, nsl],
                                 start=(dc == 0), stop=(dc == DCH - 1))
            eg = otp.tile([P, 512], f32, tag="eg")
            nc.scalar.activation(out=eg, in_=pg, func=AF.Exp, scale=-1.0)
            gt2 = otp.tile([P, 512], f32, tag="gt")
            if use_rmh:
                nc.vector.scalar_tensor_tensor(out=eg, in0=eg, scalar=1.0,
                                               in1=rmh[:, nsl],
                                               op0=ALU.add, op1=ALU.add)
                nc.vector.reciprocal_approx_fast(out=gt2, in_=eg)
            else:
                nc.vector.tensor_scalar_add(eg, eg, 1.0)
                nc.vector.reciprocal_approx_fast(out=gt2, in_=eg)
                nc.vector.tensor_mul(gt2, gt2, rmb[:, nsl])
            gts = otp.tile([64, 512], f32, tag="gts")
            nc.sync.dma_start(out=gts, in_=gt2[64:128, :])
            gate_cur[(ns, hp)] = (gt2, gts)
            if debug and "G" in debug_parts and (ns, hp) == (0, 0):
                nc.sync.dma_start(out=dbg["dbg_gt2"][:, :], in_=gt2)

        def emit_PV(g):
            it, p = divmod(g, MPAIR)
            ns, hp, hr = iters[it]
            h = 2 * hp + hr // 64
            if p == 0:
                if (ns, hp) not in gate_cur:
                    emit_gate(ns, hp)
                po_cur[0] = psO.tile([VW, 512], f32, tag="o", name="po")
            po = po_cur[0]
            need_block(b_V, (2 * p,))
            need_block(b_V, (2 * p + 1,))
            nc.tensor.matmul(po, Vp[:, p, :, h, :],
                             pts[g][:].rearrange("q (k f) -> q k f", k=2),
                             start=(p == 0), stop=(p == MPAIR - 1),
                             perf_mode=DR)
            del pts[g]
            if p == MPAIR - 1:
                # denominator sits in po row E; fast reciprocal on DVE, then
                # a replicating DMA broadcasts it to partitions 0:64
                den = stat.tile([1, 512], f32, tag="den", bufs=2)
                nc.vector.tensor_copy(out=den, in_=po[E:E + 1, :])
                rr = stat.tile([1, 512], f32, tag="rr", bufs=2)
                nc.vector.reciprocal_approx_fast(out=rr, in_=den)
                rrb = stat.tile([64, 512], f32, tag="rrb", bufs=2)
                nc.gpsimd.partition_broadcast(out_ap=rrb[:], in_ap=rr[0:1, :])
                pending_norm.append((g, it, po, rrb))
                if debug and "C" in debug_parts and it == 0:
                    pd = const.tile([P, 512], f32)
                    nc.vector.memset(pd, 0.0)
                    nc.vector.tensor_copy(out=pd[0:VW, :], in_=po)
                    nc.sync.dma_start(out=dbg["dbg_po"][:, :], in_=pd)
                    nc.sync.dma_start(out=dbg["dbg_rrb"][:, :], in_=rrb)
                po_cur[0] = None
                if it == NIT - 1:
                    emit_norm()

        def emit_norm():
            _, it, po, rrb = pending_norm.pop(0)
            ns, hp, hr = iters[it]
            nsl = slice(ns * 512, (ns + 1) * 512)
            gt2, gts = gate_cur[(ns, hp)]
            gsl = gt2[0:64, :] if hr == 0 else gts
            gr = otp.tile([64, 512], f32, tag="gr")
            nc.vector.tensor_mul(gr, gsl, rrb)
            if hr == 0:
                nc.vector.tensor_mul(OT2[0:64, hp, nsl], po[0:E, :], gr)
            else:
                tm2 = otp.tile([64, 512], CDT, tag="tm2")
                nc.vector.tensor_mul(tm2, po[0:E, :], gr)
                nc.sync.dma_start(out=OT2[64:128, hp, nsl], in_=tm2)
                del gate_cur[(ns, hp)]
            if hp == HP - 1 and hr == 64:
                emit_D(ns)

        def emit_D(ns):
            # out projection + residual + final LN (DVE-only rstd, no fence)
            t0, t1 = NTIL // NSEG * ns, NTIL // NSEG * (ns + 1)
            for nt in range(t0, t1):
                py = psO.tile([P, D], f32, tag="o", name="py")
                for c in range(DCH):
                    nc.tensor.matmul(py, OT2[:, c, nt * P:(nt + 1) * P],
                                     ow[:, c * D:(c + 1) * D],
                                     start=(c == 0), stop=(c == DCH - 1))
                yt = yt_all[:, nt, :]
                if trivial_affines:
                    nc.vector.tensor_add(yt, py, xq[:, nt, :])
                else:
                    nc.vector.tensor_add(yt, py, obias)
                    nc.vector.tensor_add(yt, yt, xq[:, nt, :])
                st2 = stat.tile([P, 6], f32, tag="st")
                nc.vector.bn_stats(out=st2, in_=yt)
                nc.vector.bn_aggr(out=mv_all[:, nt, :], in_=st2)
            rs4 = stat.tile([P, t1 - t0], f32, tag="rs4", bufs=2)
            if use_fast_rsqrt:
                fast_rsqrt(rs4, mv_all[:, t0:t1, 1], t1 - t0)
            else:
                nc.scalar.activation(out=rs4, in_=mv_all[:, t0:t1, 1],
                                     func=AF.Sqrt, bias=epsT, scale=1.0)
                nc.vector.reciprocal(out=rs4, in_=rs4)
            for nt in range(t0, t1):
                ot = otp.tile([P, D], f32, tag="fin")
                nc.vector.tensor_scalar(out=ot, in0=yt_all[:, nt, :],
                                        scalar1=mv_all[:, nt, 0:1],
                                        scalar2=rs4[:, nt - t0:nt - t0 + 1],
                                        op0=ALU.subtract, op1=ALU.mult)
                if not trivial_affines:
                    nc.vector.tensor_mul(ot, ot, gout)
                    nc.vector.tensor_add(ot, ot, bout)
                nc.sync.dma_start(out=out_ext[nt * P:(nt + 1) * P, :], in_=ot)

        for g in range(NG + L + NDELAY + 2):
            if g < NG:
                emit_S(g)
            # let the exp stream ignite before pacing leftover projections in.
            # Pace SLOWLY (1 per 3 pairs): the blocks are the only slack PE
            # work that can fill the exp-coupling gaps in the BACK half of
            # the stream -- gaps reset the PE p-state ramp and slow every
            # matmul ~25%.  Blocks an S/PV pair actually needs are pulled
            # early by need_block regardless of this pacing.
            pop_blocks(0 if g < 8 else (1 if g % 3 == 0 else 0), MCH)
            pv = g - L
            if 0 <= pv < NG:
                emit_PV(pv)
            if pending_norm and pv - pending_norm[0][0] >= NDELAY:
                emit_norm()
        while pending_norm:
            emit_norm()
        if debug and "E" in debug_parts:
            nc.sync.dma_start(out=dbg["dbg_vp"][:, :],
                              in_=Vp[:, 0].rearrange("p k h e -> p (k h e)"))
            nc.sync.dma_start(out=dbg["dbg_kt"][:, :], in_=KT2[:, 0, 0:512])
            nc.sync.dma_start(out=dbg["dbg_ot2"][:, :], in_=OT2[:, 0, 0:512])
            nc.sync.dma_start(out=dbg["dbg_yt"][:, :], in_=yt_all[:, 0, :])

    nc.finalize()
    return nc


def _prep_shared(inputs, fold_gamma_in):
    import ml_dtypes
    bf16 = ml_dtypes.bfloat16
    cos = np.asarray(inputs["rope_cos"])[:H]     # (H, E)
    sin = np.asarray(inputs["rope_sin"])[:H]

    def fold(w):
        w = np.asarray(w, np.float32)
        w1, w2 = w[..., 0::2], w[..., 1::2]
        ch = cos[:, None, 0::2].astype(np.float32)
        sh = sin[:, None, 0::2].astype(np.float32)
        out = np.empty_like(w)
        out[..., 0::2] = w1 * ch - w2 * sh
        out[..., 1::2] = w1 * sh + w2 * ch
        return out

    wstack = np.stack([fold(inputs["q_proj"]), fold(inputs["k_proj"]),
                       np.asarray(inputs["v_proj"], np.float32),
                       np.asarray(inputs["g"], np.float32)], 0)    # (4, H, D, E)
    if fold_gamma_in is not None:
        wstack = wstack * fold_gamma_in[None, None, :, None]
    wstack = wstack.reshape(4, H, DCH, P, E)
    wproj = np.ascontiguousarray(
        wstack.transpose(3, 0, 2, 1, 4)).reshape(P, 4 * DCH * HE).astype(bf16)
    # out_w (H*E, D) -> [(he)%128, (he)//128, d]
    ow = np.ascontiguousarray(
        np.asarray(inputs["out_w"], np.float32).reshape(DCH, P, D)
        .transpose(1, 0, 2)).reshape(P, DCH * D).astype(bf16)
    vecs = np.stack([inputs["gamma_in"], inputs["beta_in"],
                     inputs["gamma_out"], inputs["beta_out"],
                     inputs["out_b"]]).astype(np.float32)
    return wproj, ow, vecs


def make_in_maps(inputs, trivial_affines):
    import ml_dtypes
    x = np.asarray(inputs["x"], np.float32)
    mask = np.asarray(inputs["mask"], np.float32)
    gin = np.asarray(inputs["gamma_in"], np.float32)
    wproj, ow, vecs = _prep_shared(inputs, gin if trivial_affines else None)
    mask_bin = (mask != PAD).astype(np.float32)
    in_maps = []
    for c in range(8):
        b, j = c // 2, c % 2
        xp = np.roll(x[b], -j * NR, axis=0)
        mb = np.roll(mask_bin[b], -j * NR)
        cm_s = np.ascontiguousarray(mb.reshape(MCH, P).T)   # (P, MCH)
        rmhug = ((1.0 - mb[:NR]) * 1e30).astype(ml_dtypes.bfloat16)
        in_maps.append(dict(x=np.ascontiguousarray(xp), wproj=wproj, ow=ow,
                            vecs=vecs, cm=cm_s,
                            cmb=cm_s.astype(ml_dtypes.bfloat16),
                            rmh=rmhug,
                            rmb=mb[:NR].astype(ml_dtypes.bfloat16)))
    return in_maps


def _trivial_affines(inputs):
    return (np.all(np.asarray(inputs["beta_in"]) == 0)
            and np.all(np.asarray(inputs["gamma_out"]) == 1)
            and np.all(np.asarray(inputs["beta_out"]) == 0)
            and np.all(np.asarray(inputs["out_b"]) == 0))


def kernel(**inputs):
    from concourse.bass_utils import run_bass_kernel_spmd

    ta = _trivial_affines(inputs)
    key = ("nc", ta)
    if key not in _CACHE:
        _CACHE[key] = _build_nc(trivial_affines=ta)
    nc = _CACHE[key]

    in_maps = make_in_maps(inputs, ta)
    out = np.empty((B, N, D), np.float32)
    # Very rarely a hardware run produces NaNs (transient device flake);
    # retry a couple of times before giving up.
    for attempt in range(3):
        res = run_bass_kernel_spmd(nc, in_maps, list(range(8)))
        for c in range(8):
            b, j = c // 2, c % 2
            out[b, j * NR:(j + 1) * NR] = res.results[c]["out"]
        if np.isfinite(out).all():
            break
    return out
